# revision 13
# baseline (speedup 1.0000x reference)
"""Trainium2 Bass kernel for batch-axis-softmax dot-product attention.

Problem: B=8, S=4096, D=64 fp32.
    scores = einsum('bqd,bkd->bqk', Q, K) / 8
    attn   = softmax(scores, axis=0)          # over the BATCH axis!
    out    = einsum('bqk,bkd->bqd', attn, V)

The batch-axis softmax couples only the 8 batch entries of a fixed (q, k)
position, so sharding over the *query* axis (512 queries per core, K/V
replicated) keeps the softmax fully local to each core.

Per-core pipeline, per k-tile (128 keys x 512 queries, all 8 batches):
  PE : scoresT[k,q] = K_tile @ Q^T   (fp16, fp32 psum; batch pairs packed
       into partition halves -> row-tiled concurrent MMs)
  ACT: E_b = exp(0.125 * scores)  (one 1024-wide op per batch pair, psum src,
       written into a per-k-tile-PAIR contiguous E tile [128, 8192])
  DVE (per k-tile PAIR, to amortize per-op overhead):
       3-fold halving tree  Z = sum_b E_b        (3 wide tensor_adds)
       R = 1/Z:  either one custom-DVE reciprocal_approx_fast op (most
       pairs) or ln+exp on ACT (a tunable fraction, to balance engines)
       W = E * R   (one [128, 2, 8, 512] tensor_mul with R broadcast)
  PE : outT_b[d,q] += V_tile matmul, accumulated across all 32 k-tiles in
       persistent psum (2 batches per bank via column tiling)
Epilogue: DVE copies psum -> sbuf, DMA to HBM; host reassembles.

Scheduling: front end (scores+exp) runs per tile; the back end runs per
PAIR split into two stages (A: tree+R-start, B: R-finish+W+AV) lagged ~2
tiles so cross-engine waits are pre-satisfied when they reach each
engine's strict FIFO.
"""

import numpy as np

B = 8
S = 4096
D = 64
NCORES = 8
QBLK = S // NCORES  # 512 queries per core
KT = 128            # keys per k-tile
NKT = S // KT       # 32 k-tiles
NPAIR = B // 2      # batch pairs packed into 128 partitions
NTP = NKT // 2      # 16 k-tile pairs

# Which k-tile pairs compute R = 1/Z on ScalarE (ln+exp) instead of the
# vector engine (reciprocal_approx_fast). Balances the two engines
# (vector is the fuller engine; scalar has ~8% slack mid-kernel).
ACT_R_PAIRS = frozenset({2, 5, 8, 11})
# Tiles handled per-tile (not per-pair) at the end so the back-end drains
# right behind the exp stream instead of serializing after it.
TAIL_TILES = 4  # tiles 28..31

# test.py can flip these before calling kernel()
TRACE = False
TRACE_KWARGS = {}
LAST_RESULT = None  # BassKernelResults of the most recent run (for profiling)

_cache = {}


def _build_nc():
    from contextlib import ExitStack

    import concourse.tile as tile
    from concourse import bacc, mybir

    f16 = mybir.dt.float16
    f32 = mybir.dt.float32
    Exp = mybir.ActivationFunctionType.Exp
    Ln = mybir.ActivationFunctionType.Ln

    # Bacc (not raw Bass): its finalize() runs the legalization passes that
    # split multi-wait sync_info into EventSemaphore instructions (TRN2 allows
    # at most one wait per regular instruction).
    #
    # insert_act_table_loads maps each activation func to the first table set
    # containing it, which puts Exp in "exp_and_others" and Ln in
    # "natural_log_exp_and_others" — alternating ACT_TABLE_LOADs every k-tile
    # (~80us of ScalarE). Both funcs live in natural_log_exp_and_others, so
    # restrict Exp/Ln membership to that set: one table load for the whole
    # kernel, hoisted out of the loop.
    class _Bacc(bacc.Bacc):
        def insert_act_table_loads(self):
            from concourse import bass as bass_mod
            from concourse.hw_specs import get_activation_tables

            has_activation = any(
                isinstance(i, mybir.InstActivation)
                for b in self.main_func.blocks
                for i in b.instructions
            )
            if not has_activation:
                return
            combined = "natural_log_exp_and_others"
            tables = []
            for name, fns in get_activation_tables(self.m.arch).items():
                if name != combined:
                    fns = fns - {
                        mybir.ActivationFunctionType.Exp,
                        mybir.ActivationFunctionType.Ln,
                    }
                tables.append((name, fns))
            bass_mod._bass_rust.insert_act_table_loads(self, tables)

    nc = _Bacc()

    # Inputs pre-arranged on host into exact SBUF layouts (fp16):
    #   qt[p, j*512 + q] = Q[2j + p//64, cblk*512 + q, p%64]
    #   kt[p, j*4096 + k] = K[2j + p//64, k, p%64]
    #   vv[p, b*2048 + n*64 + d] = V[b, n*128 + p, d]
    qt_d = nc.dram_tensor("qt", [128, NPAIR * QBLK], f16, kind="ExternalInput")
    kt_d = nc.dram_tensor("kt", [128, NPAIR * S], f16, kind="ExternalInput")
    vv_d = nc.dram_tensor("vv", [128, B * NKT * D], f16, kind="ExternalInput")
    # out[j][(b%2)*64 + d, q] = out_bqd[2j + b%2, q, d]
    out_d = nc.dram_tensor("out", [NPAIR, 128, QBLK], f32, kind="ExternalOutput")

    with tile.TileContext(nc) as tc, ExitStack() as ctx:
        in_p = ctx.enter_context(tc.tile_pool(name="inp", bufs=1))
        e_p = ctx.enter_context(tc.tile_pool(name="e", bufs=3))
        w_p = ctx.enter_context(tc.tile_pool(name="w", bufs=2))
        t_p = ctx.enter_context(tc.tile_pool(name="tree", bufs=1))
        z_p = ctx.enter_context(tc.tile_pool(name="z", bufs=2))
        r_p = ctx.enter_context(tc.tile_pool(name="r", bufs=2))
        st_p = ctx.enter_context(tc.tile_pool(name="stage", bufs=1))
        ps_s = ctx.enter_context(tc.tile_pool(name="ps_s", bufs=2, space="PSUM"))
        ps_o = ctx.enter_context(tc.tile_pool(name="ps_o", bufs=1, space="PSUM"))

        # kt/vv are laid out k-tile-major on the host and DMA'd in per-tile
        # chunks interleaved kt/vv, so tile 0's operands land ~7us in and the
        # loop never waits on later chunks.
        qt = in_p.tile([128, NPAIR * QBLK], f16)
        kt = in_p.tile([128, NKT * NPAIR * KT], f16)
        vv = in_p.tile([128, NKT * B * D], f16)
        CH = NPAIR * KT  # 512 columns per k-tile chunk (for both kt and vv)

        def dma_col(dst, src, c0, c1):
            nc.sync.dma_start(out=dst[:, c0:c1], in_=src[:, c0:c1])

        # Issue order: operands of score pack (t=0, j=0) first, then the
        # rest of tile 0, then per-tile chunks so the loop never waits.
        dma_col(qt, qt_d, 0, QBLK)
        dma_col(kt, kt_d, 0, KT)          # just pack j=0 of tile 0
        dma_col(kt, kt_d, KT, CH)
        for j in range(1, NPAIR):
            dma_col(qt, qt_d, j * QBLK, (j + 1) * QBLK)
        dma_col(vv, vv_d, 0, CH)
        for t in range(1, NKT):
            dma_col(kt, kt_d, t * CH, (t + 1) * CH)
            dma_col(vv, vv_d, t * CH, (t + 1) * CH)

        # Persistent output accumulator: one 4-bank psum tile; column slice
        # j holds batches 2j (parts 0:64) and 2j+1 (parts 64:128),
        # accumulated over all 32 k-tiles. Single tile -> one wide
        # psum->sbuf copy in the epilogue instead of four.
        oacc = ps_o.tile([128, NPAIR * QBLK], f32, tag="oacc", name="oacc")

        # AV matmuls pending issue; drained between score packs so PE always
        # services the (ACT-feeding) score matmuls promptly instead of
        # running 16-MM AV bursts that starve ScalarE. Interleaving AV MMs
        # of adjacent k-tiles is safe: psum accumulate-adds commute.
        av_pending = []

        def drain_av(n):
            for _ in range(min(n, len(av_pending))):
                av_pending.pop(0)()

        # E tiles are per k-tile PAIR: [128, 8192] fp16, cols = (t%2, b, q).
        etiles = {}

        def front_end(t):
            # scores + exp for k-tile t; exp writes into the pair E tile.
            tp, r = t // 2, t % 2
            if r == 0:
                etiles[tp] = e_p.tile([128, 2 * B * QBLK], f16, tag="e", name=f"e{tp}")
            e = etiles[tp]
            for j in range(NPAIR):
                sc = ps_s.tile([128, 2 * QBLK], f32, tag="sc", name=f"sc{t}_{j}")
                for m in range(2):  # m=0 -> b=2j (rows 0:64), m=1 -> b=2j+1
                    rb = m * 64
                    nc.tensor.matmul(
                        out=sc[:, m * QBLK : (m + 1) * QBLK],
                        lhsT=kt[rb : rb + 64, t * CH + j * KT : t * CH + (j + 1) * KT],
                        rhs=qt[rb : rb + 64, j * QBLK : (j + 1) * QBLK],
                        start=True,
                        stop=True,
                        tile_position=(rb, 0),
                    )
                # E = exp(scores / sqrt(D)); scores*0.125 in [-6, 6] so no
                # max-subtraction is needed and fp16 range is safe.
                c0 = r * 4096 + j * 1024
                nc.scalar.activation(e[:, c0 : c0 + 1024], sc[:], Exp, scale=0.125)
                drain_av(2)

        zp = {}   # tp -> z tile ([128,1024] = (t%2, q)); fp16 or fp32
        r16 = {}  # tp -> R tile [128, 1024] fp16

        def recip_dve(r_ap, z_ap):
            # One custom-DVE op: ~51-ULP 1/Z, fp32 in, fp16 out.
            from concourse.dve_ops import (
                RECIP_APPROX_FAST_CONSTS,
                RECIPROCAL_APPROX_FAST,
            )

            c = RECIP_APPROX_FAST_CONSTS
            nc.vector._custom_dve(
                RECIPROCAL_APPROX_FAST,
                out=r_ap,
                in0=z_ap,
                s0=c["s0"],
                s1=c["s1"],
                imm2=c["imm2"],
            )

        def stage_a(tp):
            # Halving tree Z = sum_b E_b at pair width, then start R.
            # E cols = (t, b, q); fold1 adds b-halves {0..3}+{4..7}, fold2
            # {0,1}+{2,3} sums, fold3 the final pair -> zp (t, q).
            e = etiles[tp]
            ev = e[:].rearrange("p (t x) -> p t x", t=2)
            u1 = t_p.tile([128, 8192 // 2], f16, tag="u1", name=f"u1_{tp}")
            nc.vector.tensor_add(
                u1[:].rearrange("p (t x) -> p t x", t=2),
                ev[:, :, 0:2048],
                ev[:, :, 2048:4096],
            )
            u1v = u1[:].rearrange("p (t x) -> p t x", t=2)
            u2 = t_p.tile([128, 8192 // 4], f16, tag="u2", name=f"u2_{tp}")
            nc.vector.tensor_add(
                u2[:].rearrange("p (t x) -> p t x", t=2),
                u1v[:, :, 0:1024],
                u1v[:, :, 1024:2048],
            )
            u2v = u2[:].rearrange("p (t x) -> p t x", t=2)
            on_act = tp in ACT_R_PAIRS
            z = z_p.tile([128, 1024], f16 if on_act else f32,
                         tag="z16" if on_act else "z32",
                         name=f"z{tp}")
            nc.vector.tensor_add(
                z[:].rearrange("p (t x) -> p t x", t=2),
                u2v[:, :, 0:512],
                u2v[:, :, 512:1024],
            )
            zp[tp] = z
            r = r_p.tile([128, 1024], f16, tag="r16", name=f"r16_{tp}")
            r16[tp] = r
            if on_act:
                # R = exp(-ln(Z)) on ScalarE (shared natural_log_exp table).
                lnz = r_p.tile([128, 1024], f32, tag="lnz", name=f"lnz{tp}", bufs=1)
                nc.scalar.activation(lnz[:], z[:], Ln)
                nc.scalar.activation(r[:], lnz[:], Exp, scale=-1.0)
            else:
                recip_dve(r[:], z[:])

        def stage_b(tp):
            # W = E * R (R broadcast over b), then queue the AV matmuls.
            e = etiles.pop(tp)
            r = r16.pop(tp)
            zp.pop(tp, None)
            w = w_p.tile([128, 2 * B * QBLK], f16, tag="w", name=f"w{tp}")
            nc.vector.tensor_mul(
                w[:].rearrange("p (t b q) -> p t b q", t=2, b=B),
                e[:].rearrange("p (t b q) -> p t b q", t=2, b=B),
                r[:]
                .rearrange("p (t u q) -> p t u q", t=2, u=1)
                .to_broadcast((128, 2, B, QBLK)),
            )
            for r_ in range(2):
                emit_av(2 * tp + r_, w, r_)

        # --- per-tile tail stages (tiles NKT-TAIL_TILES .. NKT-1) ---------
        zt = {}
        rt = {}

        def tail_tree(u):
            # Halving tree + R for ONE k-tile u; E slice [128, 4096].
            # Reuses the pair-stage pool tags (half-slices) to keep SBUF flat.
            tp, r_ = u // 2, u % 2
            e = etiles[tp]
            ev = e[:, r_ * 4096 : (r_ + 1) * 4096]
            u1 = t_p.tile([128, 4096], f16, tag="u1", name=f"u1t_{u}")
            nc.vector.tensor_add(u1[:, 0:2048], ev[:, 0:2048], ev[:, 2048:4096])
            u2 = t_p.tile([128, 2048], f16, tag="u2", name=f"u2t_{u}")
            nc.vector.tensor_add(u2[:, 0:1024], u1[:, 0:1024], u1[:, 1024:2048])
            on_act = u == NKT - 1  # last tile: scalar engine is idle by then
            z = z_p.tile([128, 1024], f16 if on_act else f32,
                         tag="z16" if on_act else "z32", name=f"zt{u}")
            nc.vector.tensor_add(z[:, 0:512], u2[:, 0:512], u2[:, 512:1024])
            r = r_p.tile([128, 1024], f16, tag="r16", name=f"rt16_{u}")
            rt[u] = r
            if on_act:
                lnz = r_p.tile([128, 1024], f32, tag="lnz", name=f"lnzt{u}", bufs=1)
                nc.scalar.activation(lnz[:, 0:512], z[:, 0:512], Ln)
                nc.scalar.activation(r[:, 0:512], lnz[:, 0:512], Exp, scale=-1.0)
            else:
                recip_dve(r[:, 0:512], z[:, 0:512])

        def tail_wmult(u):
            tp, r_ = u // 2, u % 2
            e = etiles[tp]
            r = rt.pop(u)
            w = w_p.tile([128, 2 * B * QBLK], f16, tag="w", name=f"wt{u}")
            nc.vector.tensor_mul(
                w[:, 0 : B * QBLK].rearrange("p (b q) -> p b q", b=B),
                e[:, r_ * 4096 : (r_ + 1) * 4096].rearrange("p (b q) -> p b q", b=B),
                r[:, 0:512]
                .rearrange("p (u q) -> p u q", u=1)
                .to_broadcast((128, B, QBLK)),
            )
            emit_av(u, w, 0)
            if r_ == 1:
                etiles.pop(tp)

        def emit_av(t, w, r_):
            # outT_b[d,q] += V_b[t]^T-form matmul, queued for interleaved
            # issue (see drain_av). Reverse order so the first-issued AV's
            # wait (on the mult's DVE tick) covers the others.
            def mk(b):
                j, m = b // 2, b % 2
                rb = m * 64

                def go():
                    nc.tensor.matmul(
                        out=oacc[rb : rb + 64, j * QBLK : (j + 1) * QBLK],
                        lhsT=vv[:, t * CH + b * D : t * CH + (b + 1) * D],
                        rhs=w[:, r_ * 4096 + b * QBLK : r_ * 4096 + (b + 1) * QBLK],
                        start=(t == 0),
                        stop=(t == NKT - 1),
                        tile_position=(0, rb),
                        skip_group_check=True,
                    )

                return go

            for b in reversed(range(B)):
                av_pending.append(mk(b))

        # Software pipeline: front end per tile; back end per pair in two
        # stages, emitted AFTER the front end of the iter so each engine's
        # FIFO sees (exps..., back-end ops...) and cross-engine inputs are
        # produced before the consumer reaches its FIFO head.
        #   stage_a(tp) at t = 2*tp + 2, stage_b(tp) at t = 2*tp + 3
        # for tp <= NTP-3; the last TAIL_TILES tiles run per-tile stages
        # lagged one tile so the back end drains right behind the exps.
        T0 = NKT - TAIL_TILES  # first per-tile-tail tile (28)
        NPAIR_STAGES = T0 // 2  # pairs 0..13 via stage_a/b
        for t in range(NKT):
            front_end(t)
            if t >= 3 and t % 2 == 1 and (t - 3) // 2 < NPAIR_STAGES:
                stage_b((t - 3) // 2)
            if t >= 2 and t % 2 == 0 and (t - 2) // 2 < NPAIR_STAGES:
                stage_a((t - 2) // 2)
            if t > T0:
                if t - 2 >= T0:
                    tail_wmult(t - 2)
                tail_tree(t - 1)
        tail_wmult(NKT - 2)
        tail_tree(NKT - 1)
        tail_wmult(NKT - 1)
        drain_av(len(av_pending))

        # One wide staging copy (psum -> sbuf) + one output DMA (4 separate
        # dma_starts each pay ~2us of setup/completion latency in the tail).
        st = st_p.tile([128, NPAIR * QBLK], f32, tag="st")
        nc.vector.tensor_copy(out=st[:], in_=oacc[:])
        nc.sync.dma_start(
            out=out_d[:].rearrange("j p q -> p j q"),
            in_=st[:].rearrange("p (j q) -> p j q", j=NPAIR),
        )

    return nc


def _get_nc():
    if "nc" not in _cache:
        nc = _build_nc()
        if not nc.is_finalized():
            # Runs Bacc.compile() legalization (wait splitting, reg alloc).
            nc.finalize()
        _cache["nc"] = nc
    return _cache["nc"]


def _host_prep(queries, keys, values):
    """Cast to fp16 and pre-arrange into the SBUF layouts (see _build_nc)."""
    k16 = np.asarray(keys, dtype=np.float16)
    v16 = np.asarray(values, dtype=np.float16)
    q16 = np.asarray(queries, dtype=np.float16)

    # kt[(b%2)*64+d, t*512 + (b//2)*128 + kk] = K[b, t*128+kk, d] (k-tile major)
    kt = np.ascontiguousarray(
        k16.reshape(NPAIR, 2, NKT, KT, D)
        .transpose(1, 4, 2, 0, 3)
        .reshape(128, NKT * NPAIR * KT)
    )
    # vv[p, t*512 + b*64 + d] = V[b, t*128+p, d] (k-tile major)
    vv = np.ascontiguousarray(
        v16.reshape(B, NKT, KT, D).transpose(2, 1, 0, 3).reshape(128, NKT * B * D)
    )

    qts = []
    for c in range(NCORES):
        qc = q16[:, c * QBLK : (c + 1) * QBLK, :]  # [8, 512, 64]
        qt = np.ascontiguousarray(
            qc.transpose(0, 2, 1).reshape(NPAIR, 128, QBLK).transpose(1, 0, 2).reshape(128, NPAIR * QBLK)
        )
        qts.append(qt)
    return qts, kt, vv


def kernel(queries, keys, values):
    global LAST_RESULT
    from concourse.bass_utils import run_bass_kernel_spmd

    queries = np.asarray(queries, dtype=np.float32)
    keys = np.asarray(keys, dtype=np.float32)
    values = np.asarray(values, dtype=np.float32)

    nc = _get_nc()
    qts, kt, vv = _host_prep(queries, keys, values)
    in_maps = [{"qt": qts[c], "kt": kt, "vv": vv} for c in range(NCORES)]

    res = run_bass_kernel_spmd(
        nc,
        in_maps,
        list(range(NCORES)),
        trace=TRACE,
        **TRACE_KWARGS,
    )
    LAST_RESULT = res

    out = np.empty((B, S, D), dtype=np.float32)
    for c in range(NCORES):
        o = res.results[c]["out"]  # [4, 128, 512] = [j, (b%2)*64+d, q]
        out[:, c * QBLK : (c + 1) * QBLK, :] = (
            o.reshape(B, D, QBLK).transpose(0, 2, 1)
        )
    return out


# revision 14
# speedup vs baseline: 1.1625x; 1.1625x over previous
"""Trainium2 Bass kernel for batch-axis-softmax dot-product attention.

Problem: B=8, S=4096, D=64 fp32.
    scores = einsum('bqd,bkd->bqk', Q, K) / 8
    attn   = softmax(scores, axis=0)          # over the BATCH axis!
    out    = einsum('bqk,bkd->bqd', attn, V)

The batch-axis softmax couples only the 8 batch entries of a fixed (q, k)
position, so sharding over the *query* axis (512 queries per core, K/V
replicated) keeps the softmax fully local to each core.

Per-core pipeline, per k-tile (128 keys x 512 queries, all 8 batches):
  PE : scoresT[k,q] = K_tile @ Q^T   (fp16, fp32 psum; batch pairs packed
       into partition halves -> row-tiled concurrent MMs)
  ACT: E_b = exp(0.125 * scores)  (one 1024-wide op per batch pair, psum src,
       written into a per-k-tile-PAIR contiguous E tile [128, 8192])
  DVE (per k-tile PAIR, to amortize per-op overhead):
       3-fold halving tree  Z = sum_b E_b        (3 wide tensor_adds)
       R = 1/Z:  either one custom-DVE reciprocal_approx_fast op (most
       pairs) or ln+exp on ACT (a tunable fraction, to balance engines)
       W = E * R   (one [128, 2, 8, 512] tensor_mul with R broadcast)
  PE : outT_b[d,q] += V_tile matmul, accumulated across all 32 k-tiles in
       persistent psum (2 batches per bank via column tiling)
Epilogue: DVE copies psum -> sbuf, DMA to HBM; host reassembles.

Scheduling: front end (scores+exp) runs per tile; the back end runs per
PAIR split into two stages (A: tree+R-start, B: R-finish+W+AV) lagged ~2
tiles so cross-engine waits are pre-satisfied when they reach each
engine's strict FIFO.
"""

import numpy as np

B = 8
S = 4096
D = 64
NCORES = 8
QBLK = S // NCORES  # 512 queries per core
KT = 128            # keys per k-tile
NKT = S // KT       # 32 k-tiles
NPAIR = B // 2      # batch pairs packed into 128 partitions
NTP = NKT // 2      # 16 k-tile pairs

# Which k-tile pairs compute R = 1/Z on ScalarE (ln+exp) instead of the
# vector engine (reciprocal_approx_fast). Balances the two engines
# (vector is the fuller engine; scalar has slack mid-kernel).
ACT_R_PAIRS = frozenset({3, 5, 7, 9, 11, 13})
# Tiles handled per-tile (not per-pair) at the START (so the vector engine
# begins work ~one tile after the first exps instead of waiting for a full
# pair) and at the END (so the back-end drains right behind the exp
# stream instead of serializing after it).
HEAD_TILES = 4  # tiles 0..3
TAIL_TILES = 4  # tiles 28..31

# test.py can flip these before calling kernel()
TRACE = False
TRACE_KWARGS = {}
LAST_RESULT = None  # BassKernelResults of the most recent run (for profiling)

_cache = {}


def _build_nc():
    from contextlib import ExitStack

    import concourse.tile as tile
    from concourse import bacc, mybir

    f16 = mybir.dt.float16
    f32 = mybir.dt.float32
    Exp = mybir.ActivationFunctionType.Exp
    Ln = mybir.ActivationFunctionType.Ln

    # Bacc (not raw Bass): its finalize() runs the legalization passes that
    # split multi-wait sync_info into EventSemaphore instructions (TRN2 allows
    # at most one wait per regular instruction).
    #
    # insert_act_table_loads maps each activation func to the first table set
    # containing it, which puts Exp in "exp_and_others" and Ln in
    # "natural_log_exp_and_others" — alternating ACT_TABLE_LOADs every k-tile
    # (~80us of ScalarE). Both funcs live in natural_log_exp_and_others, so
    # restrict Exp/Ln membership to that set: one table load for the whole
    # kernel, hoisted out of the loop.
    class _Bacc(bacc.Bacc):
        def insert_act_table_loads(self):
            from concourse import bass as bass_mod
            from concourse.hw_specs import get_activation_tables

            has_activation = any(
                isinstance(i, mybir.InstActivation)
                for b in self.main_func.blocks
                for i in b.instructions
            )
            if not has_activation:
                return
            combined = "natural_log_exp_and_others"
            tables = []
            for name, fns in get_activation_tables(self.m.arch).items():
                if name != combined:
                    fns = fns - {
                        mybir.ActivationFunctionType.Exp,
                        mybir.ActivationFunctionType.Ln,
                    }
                tables.append((name, fns))
            bass_mod._bass_rust.insert_act_table_loads(self, tables)

    nc = _Bacc()

    # Inputs pre-arranged on host into exact SBUF layouts (fp16):
    #   qt[p, j*512 + q] = Q[2j + p//64, cblk*512 + q, p%64]
    #   kt[p, j*4096 + k] = K[2j + p//64, k, p%64]
    #   vv[p, b*2048 + n*64 + d] = V[b, n*128 + p, d]
    qt_d = nc.dram_tensor("qt", [128, NPAIR * QBLK], f16, kind="ExternalInput")
    kt_d = nc.dram_tensor("kt", [128, NPAIR * S], f16, kind="ExternalInput")
    vv_d = nc.dram_tensor("vv", [128, B * NKT * D], f16, kind="ExternalInput")
    # out[j][(b%2)*64 + d, q] = out_bqd[2j + b%2, q, d]
    out_d = nc.dram_tensor("out", [NPAIR, 128, QBLK], f32, kind="ExternalOutput")

    with tile.TileContext(nc) as tc, ExitStack() as ctx:
        in_p = ctx.enter_context(tc.tile_pool(name="inp", bufs=1))
        e_p = ctx.enter_context(tc.tile_pool(name="e", bufs=3))
        w_p = ctx.enter_context(tc.tile_pool(name="w", bufs=2))
        t_p = ctx.enter_context(tc.tile_pool(name="tree", bufs=1))
        z_p = ctx.enter_context(tc.tile_pool(name="z", bufs=2))
        r_p = ctx.enter_context(tc.tile_pool(name="r", bufs=2))
        st_p = ctx.enter_context(tc.tile_pool(name="stage", bufs=1))
        ps_s = ctx.enter_context(tc.tile_pool(name="ps_s", bufs=2, space="PSUM"))
        ps_o = ctx.enter_context(tc.tile_pool(name="ps_o", bufs=1, space="PSUM"))

        # kt/vv are laid out k-tile-major on the host and DMA'd in per-tile
        # chunks interleaved kt/vv, so tile 0's operands land ~7us in and the
        # loop never waits on later chunks.
        qt = in_p.tile([128, NPAIR * QBLK], f16)
        kt = in_p.tile([128, NKT * NPAIR * KT], f16)
        vv = in_p.tile([128, NKT * B * D], f16)
        CH = NPAIR * KT  # 512 columns per k-tile chunk (for both kt and vv)

        def dma_col(dst, src, c0, c1):
            nc.sync.dma_start(out=dst[:, c0:c1], in_=src[:, c0:c1])

        # Issue order: operands of score pack (t=0, j=0) first, then the
        # rest of tile 0, then per-tile chunks so the loop never waits.
        dma_col(qt, qt_d, 0, QBLK)
        dma_col(kt, kt_d, 0, KT)          # just pack j=0 of tile 0
        dma_col(kt, kt_d, KT, CH)
        for j in range(1, NPAIR):
            dma_col(qt, qt_d, j * QBLK, (j + 1) * QBLK)
        dma_col(vv, vv_d, 0, CH)
        for t in range(1, NKT):
            dma_col(kt, kt_d, t * CH, (t + 1) * CH)
            dma_col(vv, vv_d, t * CH, (t + 1) * CH)

        # Persistent output accumulator: one 4-bank psum tile; column slice
        # j holds batches 2j (parts 0:64) and 2j+1 (parts 64:128),
        # accumulated over all 32 k-tiles. Single tile -> one wide
        # psum->sbuf copy in the epilogue instead of four.
        oacc = ps_o.tile([128, NPAIR * QBLK], f32, tag="oacc", name="oacc")

        # AV matmuls pending issue; drained between score packs so PE always
        # services the (ACT-feeding) score matmuls promptly instead of
        # running 16-MM AV bursts that starve ScalarE. Interleaving AV MMs
        # of adjacent k-tiles is safe: psum accumulate-adds commute.
        av_pending = []

        def drain_av(n):
            for _ in range(min(n, len(av_pending))):
                av_pending.pop(0)()

        # E tiles are per k-tile PAIR: [128, 8192] fp16, cols = (t%2, b, q).
        etiles = {}

        def front_end(t):
            # scores + exp for k-tile t; exp writes into the pair E tile.
            tp, r = t // 2, t % 2
            if r == 0:
                etiles[tp] = e_p.tile([128, 2 * B * QBLK], f16, tag="e", name=f"e{tp}")
            e = etiles[tp]
            for j in range(NPAIR):
                sc = ps_s.tile([128, 2 * QBLK], f32, tag="sc", name=f"sc{t}_{j}")
                for m in range(2):  # m=0 -> b=2j (rows 0:64), m=1 -> b=2j+1
                    rb = m * 64
                    nc.tensor.matmul(
                        out=sc[:, m * QBLK : (m + 1) * QBLK],
                        lhsT=kt[rb : rb + 64, t * CH + j * KT : t * CH + (j + 1) * KT],
                        rhs=qt[rb : rb + 64, j * QBLK : (j + 1) * QBLK],
                        start=True,
                        stop=True,
                        tile_position=(rb, 0),
                    )
                # E = exp(scores / sqrt(D)); scores*0.125 in [-6, 6] so no
                # max-subtraction is needed and fp16 range is safe.
                c0 = r * 4096 + j * 1024
                nc.scalar.activation(e[:, c0 : c0 + 1024], sc[:], Exp, scale=0.125)
                drain_av(2)

        zp = {}   # tp -> z tile ([128,1024] = (t%2, q)); fp16 or fp32
        r16 = {}  # tp -> R tile [128, 1024] fp16

        def recip_dve(r_ap, z_ap):
            # One custom-DVE op: ~51-ULP 1/Z, fp32 in, fp16 out.
            from concourse.dve_ops import (
                RECIP_APPROX_FAST_CONSTS,
                RECIPROCAL_APPROX_FAST,
            )

            c = RECIP_APPROX_FAST_CONSTS
            nc.vector._custom_dve(
                RECIPROCAL_APPROX_FAST,
                out=r_ap,
                in0=z_ap,
                s0=c["s0"],
                s1=c["s1"],
                imm2=c["imm2"],
            )

        def stage_a(tp):
            # Halving tree Z = sum_b E_b at pair width, then start R.
            # E cols = (t, b, q); fold1 adds b-halves {0..3}+{4..7}, fold2
            # {0,1}+{2,3} sums, fold3 the final pair -> zp (t, q).
            e = etiles[tp]
            ev = e[:].rearrange("p (t x) -> p t x", t=2)
            u1 = t_p.tile([128, 8192 // 2], f16, tag="u1", name=f"u1_{tp}")
            nc.vector.tensor_add(
                u1[:].rearrange("p (t x) -> p t x", t=2),
                ev[:, :, 0:2048],
                ev[:, :, 2048:4096],
            )
            u1v = u1[:].rearrange("p (t x) -> p t x", t=2)
            u2 = t_p.tile([128, 8192 // 4], f16, tag="u2", name=f"u2_{tp}")
            nc.vector.tensor_add(
                u2[:].rearrange("p (t x) -> p t x", t=2),
                u1v[:, :, 0:1024],
                u1v[:, :, 1024:2048],
            )
            u2v = u2[:].rearrange("p (t x) -> p t x", t=2)
            on_act = tp in ACT_R_PAIRS
            z = z_p.tile([128, 1024], f16 if on_act else f32,
                         tag="z16" if on_act else "z32",
                         name=f"z{tp}")
            nc.vector.tensor_add(
                z[:].rearrange("p (t x) -> p t x", t=2),
                u2v[:, :, 0:512],
                u2v[:, :, 512:1024],
            )
            zp[tp] = z
            r = r_p.tile([128, 1024], f16, tag="r16", name=f"r16_{tp}")
            r16[tp] = r
            if on_act:
                # R = exp(-ln(Z)) on ScalarE (shared natural_log_exp table).
                lnz = r_p.tile([128, 1024], f32, tag="lnz", name=f"lnz{tp}", bufs=1)
                nc.scalar.activation(lnz[:], z[:], Ln)
                nc.scalar.activation(r[:], lnz[:], Exp, scale=-1.0)
            else:
                recip_dve(r[:], z[:])

        def stage_b(tp):
            # W = E * R (R broadcast over b), then queue the AV matmuls.
            e = etiles.pop(tp)
            r = r16.pop(tp)
            zp.pop(tp, None)
            w = w_p.tile([128, 2 * B * QBLK], f16, tag="w", name=f"w{tp}")
            nc.vector.tensor_mul(
                w[:].rearrange("p (t b q) -> p t b q", t=2, b=B),
                e[:].rearrange("p (t b q) -> p t b q", t=2, b=B),
                r[:]
                .rearrange("p (t u q) -> p t u q", t=2, u=1)
                .to_broadcast((128, 2, B, QBLK)),
            )
            for r_ in range(2):
                emit_av(2 * tp + r_, w, r_)

        # --- per-tile tail stages (tiles NKT-TAIL_TILES .. NKT-1) ---------
        zt = {}
        rt = {}

        def tail_tree(u):
            # Halving tree + R for ONE k-tile u; E slice [128, 4096].
            # Reuses the pair-stage pool tags (half-slices) to keep SBUF flat.
            tp, r_ = u // 2, u % 2
            e = etiles[tp]
            ev = e[:, r_ * 4096 : (r_ + 1) * 4096]
            u1 = t_p.tile([128, 4096], f16, tag="u1", name=f"u1t_{u}")
            nc.vector.tensor_add(u1[:, 0:2048], ev[:, 0:2048], ev[:, 2048:4096])
            u2 = t_p.tile([128, 2048], f16, tag="u2", name=f"u2t_{u}")
            nc.vector.tensor_add(u2[:, 0:1024], u1[:, 0:1024], u1[:, 1024:2048])
            on_act = u == NKT - 1  # last tile: scalar engine is idle by then
            z = z_p.tile([128, 1024], f16 if on_act else f32,
                         tag="z16" if on_act else "z32", name=f"zt{u}")
            nc.vector.tensor_add(z[:, 0:512], u2[:, 0:512], u2[:, 512:1024])
            r = r_p.tile([128, 1024], f16, tag="r16", name=f"rt16_{u}")
            rt[u] = r
            if on_act:
                lnz = r_p.tile([128, 1024], f32, tag="lnz", name=f"lnzt{u}", bufs=1)
                nc.scalar.activation(lnz[:, 0:512], z[:, 0:512], Ln)
                nc.scalar.activation(r[:, 0:512], lnz[:, 0:512], Exp, scale=-1.0)
            else:
                recip_dve(r[:, 0:512], z[:, 0:512])

        def tail_wmult(u):
            tp, r_ = u // 2, u % 2
            e = etiles[tp]
            r = rt.pop(u)
            w = w_p.tile([128, 2 * B * QBLK], f16, tag="w", name=f"wt{u}")
            nc.vector.tensor_mul(
                w[:, 0 : B * QBLK].rearrange("p (b q) -> p b q", b=B),
                e[:, r_ * 4096 : (r_ + 1) * 4096].rearrange("p (b q) -> p b q", b=B),
                r[:, 0:512]
                .rearrange("p (u q) -> p u q", u=1)
                .to_broadcast((128, B, QBLK)),
            )
            emit_av(u, w, 0)
            if r_ == 1:
                etiles.pop(tp)

        def emit_av(t, w, r_):
            # outT_b[d,q] += V_b[t]^T-form matmul, queued for interleaved
            # issue (see drain_av). Reverse order so the first-issued AV's
            # wait (on the mult's DVE tick) covers the others.
            def mk(b):
                j, m = b // 2, b % 2
                rb = m * 64

                def go():
                    nc.tensor.matmul(
                        out=oacc[rb : rb + 64, j * QBLK : (j + 1) * QBLK],
                        lhsT=vv[:, t * CH + b * D : t * CH + (b + 1) * D],
                        rhs=w[:, r_ * 4096 + b * QBLK : r_ * 4096 + (b + 1) * QBLK],
                        start=(t == 0),
                        stop=(t == NKT - 1),
                        tile_position=(0, rb),
                        skip_group_check=True,
                    )

                return go

            for b in reversed(range(B)):
                av_pending.append(mk(b))

        # Software pipeline: front end per tile; back end per pair in two
        # stages, emitted AFTER the front end of the iter so each engine's
        # FIFO sees (exps..., back-end ops...) and cross-engine inputs are
        # produced before the consumer reaches its FIFO head.
        #   stage_a(tp) at t = 2*tp + 2, stage_b(tp) at t = 2*tp + 3
        # for tp <= NTP-3; the last TAIL_TILES tiles run per-tile stages
        # lagged one tile so the back end drains right behind the exps.
        T0 = NKT - TAIL_TILES  # first per-tile-tail tile (28)
        NPAIR_STAGES = T0 // 2  # pairs 0..13 via stage_a/b
        for t in range(NKT):
            front_end(t)
            if t >= 3 and t % 2 == 1 and (t - 3) // 2 < NPAIR_STAGES:
                stage_b((t - 3) // 2)
            if t >= 2 and t % 2 == 0 and (t - 2) // 2 < NPAIR_STAGES:
                stage_a((t - 2) // 2)
            if t > T0:
                if t - 2 >= T0:
                    tail_wmult(t - 2)
                tail_tree(t - 1)
        tail_wmult(NKT - 2)
        tail_tree(NKT - 1)
        tail_wmult(NKT - 1)
        drain_av(len(av_pending))

        # One wide staging copy (psum -> sbuf) + one output DMA (4 separate
        # dma_starts each pay ~2us of setup/completion latency in the tail).
        st = st_p.tile([128, NPAIR * QBLK], f32, tag="st")
        nc.vector.tensor_copy(out=st[:], in_=oacc[:])
        nc.sync.dma_start(
            out=out_d[:].rearrange("j p q -> p j q"),
            in_=st[:].rearrange("p (j q) -> p j q", j=NPAIR),
        )

    return nc


def _get_nc():
    if "nc" not in _cache:
        nc = _build_nc()
        if not nc.is_finalized():
            # Runs Bacc.compile() legalization (wait splitting, reg alloc).
            nc.finalize()
        _cache["nc"] = nc
    return _cache["nc"]


def _host_prep(queries, keys, values):
    """Cast to fp16 and pre-arrange into the SBUF layouts (see _build_nc)."""
    k16 = np.asarray(keys, dtype=np.float16)
    v16 = np.asarray(values, dtype=np.float16)
    q16 = np.asarray(queries, dtype=np.float16)

    # kt[(b%2)*64+d, t*512 + (b//2)*128 + kk] = K[b, t*128+kk, d] (k-tile major)
    kt = np.ascontiguousarray(
        k16.reshape(NPAIR, 2, NKT, KT, D)
        .transpose(1, 4, 2, 0, 3)
        .reshape(128, NKT * NPAIR * KT)
    )
    # vv[p, t*512 + b*64 + d] = V[b, t*128+p, d] (k-tile major)
    vv = np.ascontiguousarray(
        v16.reshape(B, NKT, KT, D).transpose(2, 1, 0, 3).reshape(128, NKT * B * D)
    )

    qts = []
    for c in range(NCORES):
        qc = q16[:, c * QBLK : (c + 1) * QBLK, :]  # [8, 512, 64]
        qt = np.ascontiguousarray(
            qc.transpose(0, 2, 1).reshape(NPAIR, 128, QBLK).transpose(1, 0, 2).reshape(128, NPAIR * QBLK)
        )
        qts.append(qt)
    return qts, kt, vv


def kernel(queries, keys, values):
    global LAST_RESULT
    from concourse.bass_utils import run_bass_kernel_spmd

    queries = np.asarray(queries, dtype=np.float32)
    keys = np.asarray(keys, dtype=np.float32)
    values = np.asarray(values, dtype=np.float32)

    nc = _get_nc()
    qts, kt, vv = _host_prep(queries, keys, values)
    in_maps = [{"qt": qts[c], "kt": kt, "vv": vv} for c in range(NCORES)]

    res = run_bass_kernel_spmd(
        nc,
        in_maps,
        list(range(NCORES)),
        trace=TRACE,
        **TRACE_KWARGS,
    )
    LAST_RESULT = res

    out = np.empty((B, S, D), dtype=np.float32)
    for c in range(NCORES):
        o = res.results[c]["out"]  # [4, 128, 512] = [j, (b%2)*64+d, q]
        out[:, c * QBLK : (c + 1) * QBLK, :] = (
            o.reshape(B, D, QBLK).transpose(0, 2, 1)
        )
    return out


# revision 38
# speedup vs baseline: 1.2862x; 1.1064x over previous
"""Trainium2 Bass kernel for batch-axis-softmax dot-product attention.

Problem: B=8, S=4096, D=64 fp32.
    scores = einsum('bqd,bkd->bqk', Q, K) / 8
    attn   = softmax(scores, axis=0)          # over the BATCH axis!
    out    = einsum('bqk,bkd->bqd', attn, V)

The batch-axis softmax couples only the 8 batch entries of a fixed (q, k)
position, so sharding over the *query* axis (512 queries per core, K/V
replicated) keeps the softmax fully local to each core.

Per-core pipeline, per k-tile (128 keys x 512 queries, all 8 batches):
  PE : scoresT[k,q] = K_tile @ Q^T   (fp16, fp32 psum; batch pairs packed
       into partition halves -> row-tiled concurrent MMs)
  ACT: E_b = exp(0.125 * scores)  (one 1024-wide op per batch pair, psum src,
       written into a per-k-tile-PAIR contiguous E tile [128, 8192])
  DVE (per k-tile PAIR, to amortize per-op overhead):
       3-fold halving tree  Z = sum_b E_b        (3 wide tensor_adds)
       R = 1/Z:  either one custom-DVE reciprocal_approx_fast op (most
       pairs) or ln+exp on ACT (a tunable fraction, to balance engines)
       W = E * R   (one [128, 2, 8, 512] tensor_mul with R broadcast)
  PE : outT_b[d,q] += V_tile matmul, accumulated across all 32 k-tiles in
       persistent psum (2 batches per bank via column tiling)
Epilogue: DVE copies psum -> sbuf, DMA to HBM; host reassembles.

Scheduling: front end (scores+exp) runs per tile; the back end runs per
PAIR in three stages (tree @2tp+1, ACT-R @2tp+2, W+AV @2tp+4) so every
cross-engine input is ready before the consumer reaches it in its
engine's strict FIFO; the first/last four tiles run per-tile stages so
the vector engine starts right behind the first exps and drains right
behind the last ones. Steady state: ScalarE ~148us (128 exps at the
(172+1024)/1.2GHz floor + R share), DVE ~153us. HW exec ~184us on 8
cores (baseline for this problem: ~202us; first working version 445us).
"""

import numpy as np

B = 8
S = 4096
D = 64
NCORES = 8
QBLK = S // NCORES  # 512 queries per core
KT = 128            # keys per k-tile
NKT = S // KT       # 32 k-tiles
NPAIR = B // 2      # batch pairs packed into 128 partitions
NTP = NKT // 2      # 16 k-tile pairs

# Which k-tile pairs compute R = 1/Z on ScalarE (ln+exp) instead of the
# vector engine (reciprocal_approx_fast). Balances the two engines
# (vector is the fuller engine; scalar has slack mid-kernel).
# First ACT pair no earlier than tp=5: in the warm-up phase the vector
# engine's queue is shallow, and an early ACT-R round-trip still bubbles.
ACT_R_PAIRS = frozenset({5, 7, 9, 11, 12, 13})
# Tiles handled per-tile (not per-pair) at the START (so the vector engine
# begins work ~one tile after the first exps instead of waiting for a full
# pair) and at the END (so the back-end drains right behind the exp
# stream instead of serializing after it).
HEAD_TILES = 4  # tiles 0..3
TAIL_TILES = 4  # tiles 28..31

# test.py can flip these before calling kernel()
TRACE = False
TRACE_KWARGS = {}
LAST_RESULT = None  # BassKernelResults of the most recent run (for profiling)

_cache = {}


def _build_nc():
    from contextlib import ExitStack

    import concourse.tile as tile
    from concourse import bacc, mybir

    f16 = mybir.dt.float16
    f32 = mybir.dt.float32
    Exp = mybir.ActivationFunctionType.Exp
    Ln = mybir.ActivationFunctionType.Ln

    # Bacc (not raw Bass): its finalize() runs the legalization passes that
    # split multi-wait sync_info into EventSemaphore instructions (TRN2 allows
    # at most one wait per regular instruction).
    #
    # insert_act_table_loads maps each activation func to the first table set
    # containing it, which puts Exp in "exp_and_others" and Ln in
    # "natural_log_exp_and_others" — alternating ACT_TABLE_LOADs every k-tile
    # (~80us of ScalarE). Both funcs live in natural_log_exp_and_others, so
    # restrict Exp/Ln membership to that set: one table load for the whole
    # kernel, hoisted out of the loop.
    class _Bacc(bacc.Bacc):
        def insert_act_table_loads(self):
            from concourse import bass as bass_mod
            from concourse.hw_specs import get_activation_tables

            has_activation = any(
                isinstance(i, mybir.InstActivation)
                for b in self.main_func.blocks
                for i in b.instructions
            )
            if not has_activation:
                return
            combined = "natural_log_exp_and_others"
            tables = []
            for name, fns in get_activation_tables(self.m.arch).items():
                if name != combined:
                    fns = fns - {
                        mybir.ActivationFunctionType.Exp,
                        mybir.ActivationFunctionType.Ln,
                    }
                tables.append((name, fns))
            bass_mod._bass_rust.insert_act_table_loads(self, tables)

    nc = _Bacc()

    # Inputs pre-arranged on host into exact SBUF layouts (fp16):
    #   qt[p, j*512 + q] = Q[2j + p//64, cblk*512 + q, p%64]
    #   kt[p, j*4096 + k] = K[2j + p//64, k, p%64]
    #   vv[p, b*2048 + n*64 + d] = V[b, n*128 + p, d]
    qt_d = nc.dram_tensor("qt", [128, NPAIR * QBLK], f16, kind="ExternalInput")
    kt_d = nc.dram_tensor("kt", [128, NPAIR * S], f16, kind="ExternalInput")
    vv_d = nc.dram_tensor("vv", [128, B * NKT * D], f16, kind="ExternalInput")
    # out[j][(b%2)*64 + d, q] = out_bqd[2j + b%2, q, d]
    out_d = nc.dram_tensor("out", [NPAIR, 128, QBLK], f32, kind="ExternalOutput")

    with tile.TileContext(nc) as tc, ExitStack() as ctx:
        in_p = ctx.enter_context(tc.tile_pool(name="inp", bufs=1))
        e_p = ctx.enter_context(tc.tile_pool(name="e", bufs=4))
        w_p = ctx.enter_context(tc.tile_pool(name="w", bufs=2))
        t_p = ctx.enter_context(tc.tile_pool(name="tree", bufs=1))
        z_p = ctx.enter_context(tc.tile_pool(name="z", bufs=2))
        r_p = ctx.enter_context(tc.tile_pool(name="r", bufs=2))
        st_p = ctx.enter_context(tc.tile_pool(name="stage", bufs=1))
        ps_s = ctx.enter_context(tc.tile_pool(name="ps_s", bufs=2, space="PSUM"))
        ps_o = ctx.enter_context(tc.tile_pool(name="ps_o", bufs=1, space="PSUM"))

        # kt/vv are laid out k-tile-major on the host and DMA'd in per-tile
        # chunks interleaved kt/vv, so tile 0's operands land ~7us in and the
        # loop never waits on later chunks.
        qt = in_p.tile([128, NPAIR * QBLK], f16)
        kt = in_p.tile([128, NKT * NPAIR * KT], f16)
        vv = in_p.tile([128, NKT * B * D], f16)
        CH = NPAIR * KT  # 512 columns per k-tile chunk (for both kt and vv)

        def dma_col(dst, src, c0, c1):
            nc.sync.dma_start(out=dst[:, c0:c1], in_=src[:, c0:c1])

        # Issue order: operands of score pack (t=0, j=0) first, then the
        # rest of tile 0, then per-tile chunks so the loop never waits.
        dma_col(qt, qt_d, 0, QBLK)
        dma_col(kt, kt_d, 0, KT)          # just pack j=0 of tile 0
        dma_col(kt, kt_d, KT, CH)
        for j in range(1, NPAIR):
            dma_col(qt, qt_d, j * QBLK, (j + 1) * QBLK)
        dma_col(vv, vv_d, 0, CH)
        for t in range(1, NKT):
            dma_col(kt, kt_d, t * CH, (t + 1) * CH)
            dma_col(vv, vv_d, t * CH, (t + 1) * CH)

        # Persistent output accumulators: bank j holds batches 2j (parts
        # 0:64) and 2j+1 (parts 64:128), accumulated over all 32 k-tiles.
        oacc = [
            ps_o.tile([128, QBLK], f32, tag=f"oacc{j}", name=f"oacc{j}")
            for j in range(NPAIR)
        ]

        # AV matmuls pending issue; drained between score packs so PE always
        # services the (ACT-feeding) score matmuls promptly instead of
        # running 16-MM AV bursts that starve ScalarE. Interleaving AV MMs
        # of adjacent k-tiles is safe: psum accumulate-adds commute.
        av_pending = []

        def drain_av(n):
            for _ in range(min(n, len(av_pending))):
                av_pending.pop(0)()

        # E tiles are per k-tile PAIR: [128, 8192] fp16, cols = (t%2, b, q).
        etiles = {}

        def front_end(t):
            # scores + exp for k-tile t; exp writes into the pair E tile.
            tp, r = t // 2, t % 2
            if r == 0:
                etiles[tp] = e_p.tile([128, 2 * B * QBLK], f16, tag="e", name=f"e{tp}")
            e = etiles[tp]
            for j in range(NPAIR):
                sc = ps_s.tile([128, 2 * QBLK], f32, tag="sc", name=f"sc{t}_{j}")
                for m in range(2):  # m=0 -> b=2j (rows 0:64), m=1 -> b=2j+1
                    rb = m * 64
                    nc.tensor.matmul(
                        out=sc[:, m * QBLK : (m + 1) * QBLK],
                        lhsT=kt[rb : rb + 64, t * CH + j * KT : t * CH + (j + 1) * KT],
                        rhs=qt[rb : rb + 64, j * QBLK : (j + 1) * QBLK],
                        start=True,
                        stop=True,
                        tile_position=(rb, 0),
                    )
                # E = exp(scores / sqrt(D)); scores*0.125 in [-6, 6] so no
                # max-subtraction is needed and fp16 range is safe.
                c0 = r * 4096 + j * 1024
                nc.scalar.activation(e[:, c0 : c0 + 1024], sc[:], Exp, scale=0.125)
                drain_av(2)

        zp = {}   # tp -> z tile ([128,1024] = (t%2, q)); fp16 or fp32
        r16 = {}  # tp -> R tile [128, 1024] fp16

        def recip_dve(r_ap, z_ap):
            # One custom-DVE op: ~51-ULP 1/Z, fp32 in, fp16 out.
            from concourse.dve_ops import (
                RECIP_APPROX_FAST_CONSTS,
                RECIPROCAL_APPROX_FAST,
            )

            c = RECIP_APPROX_FAST_CONSTS
            nc.vector._custom_dve(
                RECIPROCAL_APPROX_FAST,
                out=r_ap,
                in0=z_ap,
                s0=c["s0"],
                s1=c["s1"],
                imm2=c["imm2"],
            )

        def stage_a(tp):
            # Halving tree Z = sum_b E_b at pair width, then start R.
            # E cols = (t, b, q); fold1 adds b-halves {0..3}+{4..7}, fold2
            # {0,1}+{2,3} sums, fold3 the final pair -> zp (t, q).
            e = etiles[tp]
            ev = e[:].rearrange("p (t x) -> p t x", t=2)
            u1 = t_p.tile([128, 8192 // 2], f16, tag="u1", name=f"u1_{tp}")
            nc.vector.tensor_add(
                u1[:].rearrange("p (t x) -> p t x", t=2),
                ev[:, :, 0:2048],
                ev[:, :, 2048:4096],
            )
            u1v = u1[:].rearrange("p (t x) -> p t x", t=2)
            u2 = t_p.tile([128, 8192 // 4], f16, tag="u2", name=f"u2_{tp}")
            nc.vector.tensor_add(
                u2[:].rearrange("p (t x) -> p t x", t=2),
                u1v[:, :, 0:1024],
                u1v[:, :, 1024:2048],
            )
            u2v = u2[:].rearrange("p (t x) -> p t x", t=2)
            on_act = tp in ACT_R_PAIRS
            # fp16 z in both paths: the DVE converts reads to fp32 before
            # the ALU stages, so reciprocal_approx_fast's BITWISE_NOT seed
            # sees proper fp32 bits even from an fp16 source; fold3 keeps
            # its 2x-packed mode this way.
            z = z_p.tile([128, 1024], f16, tag="z16", name=f"z{tp}")
            nc.vector.tensor_add(
                z[:].rearrange("p (t x) -> p t x", t=2),
                u2v[:, :, 0:512],
                u2v[:, :, 512:1024],
            )
            zp[tp] = z
            r = r_p.tile([128, 1024], f16, tag="r16", name=f"r16_{tp}", bufs=3)
            r16[tp] = r
            if not on_act:
                recip_dve(r[:], z[:])

        def stage_r(tp):
            # R = exp(-ln(Z)) on ScalarE (shared natural_log_exp table),
            # emitted one tile AFTER the tree so the LN sits behind that
            # tile's exps in the scalar FIFO and never stalls the stream.
            z = zp[tp]
            r = r16[tp]
            lnz = r_p.tile([128, 1024], f32, tag="lnz", name=f"lnz{tp}", bufs=1)
            nc.scalar.activation(lnz[:], z[:], Ln)
            nc.scalar.activation(r[:], lnz[:], Exp, scale=-1.0)

        def stage_b(tp):
            # W = E * R (R broadcast over b), then queue the AV matmuls.
            e = etiles.pop(tp)
            r = r16.pop(tp)
            zp.pop(tp, None)
            w = w_p.tile([128, 2 * B * QBLK], f16, tag="w", name=f"w{tp}")
            nc.vector.tensor_mul(
                w[:].rearrange("p (t b q) -> p t b q", t=2, b=B),
                e[:].rearrange("p (t b q) -> p t b q", t=2, b=B),
                r[:]
                .rearrange("p (t u q) -> p t u q", t=2, u=1)
                .to_broadcast((128, 2, B, QBLK)),
            )
            for r_ in range(2):
                emit_av(2 * tp + r_, w, r_)

        # --- per-tile stages (head tiles 0..HEAD_TILES-1 and the last
        # TAIL_TILES tiles) ------------------------------------------------
        rt = {}

        def tail_tree(u):
            # Halving tree + R for ONE k-tile u; E slice [128, 4096].
            # Reuses the pair-stage pool tags (half-slices) to keep SBUF flat.
            tp, r_ = u // 2, u % 2
            e = etiles[tp]
            ev = e[:, r_ * 4096 : (r_ + 1) * 4096]
            u1 = t_p.tile([128, 4096], f16, tag="u1", name=f"u1t_{u}")
            nc.vector.tensor_add(u1[:, 0:2048], ev[:, 0:2048], ev[:, 2048:4096])
            u2 = t_p.tile([128, 2048], f16, tag="u2", name=f"u2t_{u}")
            nc.vector.tensor_add(u2[:, 0:1024], u1[:, 0:1024], u1[:, 1024:2048])
            on_act = u == NKT - 1  # last tile: scalar engine is idle by then
            z = z_p.tile([128, 1024], f16, tag="z16", name=f"zt{u}")
            nc.vector.tensor_add(z[:, 0:512], u2[:, 0:512], u2[:, 512:1024])
            r = r_p.tile([128, 1024], f16, tag="r16", name=f"rt16_{u}", bufs=3)
            rt[u] = r
            if on_act:
                lnz = r_p.tile([128, 1024], f32, tag="lnz", name=f"lnzt{u}", bufs=1)
                nc.scalar.activation(lnz[:, 0:512], z[:, 0:512], Ln)
                nc.scalar.activation(r[:, 0:512], lnz[:, 0:512], Exp, scale=-1.0)
            else:
                recip_dve(r[:, 0:512], z[:, 0:512])

        def tail_wmult(u):
            if u >= T0:
                drain_av(16)  # flush backlog so the w pool recycles promptly
            tp, r_ = u // 2, u % 2
            e = etiles[tp]
            r = rt.pop(u)
            w = w_p.tile([128, 2 * B * QBLK], f16, tag="w", name=f"wt{u}")
            nc.vector.tensor_mul(
                w[:, 0 : B * QBLK].rearrange("p (b q) -> p b q", b=B),
                e[:, r_ * 4096 : (r_ + 1) * 4096].rearrange("p (b q) -> p b q", b=B),
                r[:, 0:512]
                .rearrange("p (u q) -> p u q", u=1)
                .to_broadcast((128, B, QBLK)),
            )
            emit_av(u, w, 0)
            if r_ == 1:
                etiles.pop(tp)

        def emit_av(t, w, r_):
            # outT_b[d,q] += V_b[t]^T-form matmul, queued for interleaved
            # issue (see drain_av). Reverse order so the first-issued AV's
            # wait (on the mult's DVE tick) covers the others.
            def mk(b):
                j, m = b // 2, b % 2
                rb = m * 64

                def go():
                    nc.tensor.matmul(
                        out=oacc[j][rb : rb + 64, :],
                        lhsT=vv[:, t * CH + b * D : t * CH + (b + 1) * D],
                        rhs=w[:, r_ * 4096 + b * QBLK : r_ * 4096 + (b + 1) * QBLK],
                        start=(t == 0),
                        stop=(t == NKT - 1),
                        tile_position=(0, rb),
                        skip_group_check=True,
                    )

                return go

            for b in reversed(range(B)):
                av_pending.append(mk(b))

        # Software pipeline: front end per tile; back end per pair in three
        # stages, emitted AFTER the front end of the iter so each engine's
        # FIFO sees (exps..., back-end ops...) in stream order:
        #   stage_a(tp)  @ t=2tp+1 (odd):  tree [+recip for DVE-R pairs]
        #   stage_r(tp)  @ t=2tp+2 (even): ln+exp on ScalarE (ACT-R pairs)
        #   stage_b(tp)  @ t=2tp+4 (even): W-mult + AV queue. The 2-iter lag
        #     means R is always ready before the vector FIFO reaches the
        #     wmult, so DVE alternates [folds] / [wmult] every iter and
        #     never stalls on an ACT-R pair's ln+exp round-trip.
        # Head tiles (0..HEAD_TILES-1) and tail tiles run per-tile stages
        # lagged one tile (tree(u) after front(u+1), wmult(u) one later).
        T0 = NKT - TAIL_TILES     # first per-tile-tail tile (28)
        PAIR_LO = HEAD_TILES // 2  # first pair handled by stage_a/r/b (2)
        NPAIR_STAGES = T0 // 2     # pairs PAIR_LO..13 via stage_a/r/b
        for t in range(NKT):
            front_end(t)
            if 2 <= t < HEAD_TILES + 2:
                tail_wmult(t - 2)
            if 1 <= t < HEAD_TILES + 1:
                tail_tree(t - 1)
            if t % 2 == 1:
                tp_t = (t - 1) // 2
                if PAIR_LO <= tp_t < NPAIR_STAGES:
                    stage_a(tp_t)
            else:
                tp_r = (t - 2) // 2
                if PAIR_LO <= tp_r < NPAIR_STAGES and tp_r in ACT_R_PAIRS:
                    stage_r(tp_r)
                tp_w = (t - 4) // 2
                if PAIR_LO <= tp_w < NPAIR_STAGES:
                    stage_b(tp_w)
            if t > T0:
                if t - 2 >= T0:
                    tail_wmult(t - 2)
                tail_tree(t - 1)
        # tree(31) before wmult(30): its R runs on the (idle) scalar engine
        # concurrently with wmult(30) on vector, so wmult(31) never waits.
        tail_tree(NKT - 1)
        tail_wmult(NKT - 2)
        tail_wmult(NKT - 1)
        drain_av(len(av_pending))

        # Per-bank psum -> sbuf copies (each can start as soon as that
        # bank's last AV lands) + per-bank output DMAs so the last DMA
        # only moves 256KB instead of the full 1MB.
        # j=3..0: the AV queue drains b=7..0, so oacc3's last AV lands first.
        st = st_p.tile([128, NPAIR * QBLK], f32, tag="st")
        for j in reversed(range(NPAIR)):
            nc.vector.tensor_copy(
                out=st[:, j * QBLK : (j + 1) * QBLK], in_=oacc[j][:]
            )
            nc.sync.dma_start(
                out=out_d[j, :, :],
                in_=st[:, j * QBLK : (j + 1) * QBLK],
            )

    return nc


def _get_nc():
    if "nc" not in _cache:
        nc = _build_nc()
        if not nc.is_finalized():
            # Runs Bacc.compile() legalization (wait splitting, reg alloc).
            nc.finalize()
        _cache["nc"] = nc
    return _cache["nc"]


def _host_prep(queries, keys, values):
    """Cast to fp16 and pre-arrange into the SBUF layouts (see _build_nc)."""
    k16 = np.asarray(keys, dtype=np.float16)
    v16 = np.asarray(values, dtype=np.float16)
    q16 = np.asarray(queries, dtype=np.float16)

    # kt[(b%2)*64+d, t*512 + (b//2)*128 + kk] = K[b, t*128+kk, d] (k-tile major)
    kt = np.ascontiguousarray(
        k16.reshape(NPAIR, 2, NKT, KT, D)
        .transpose(1, 4, 2, 0, 3)
        .reshape(128, NKT * NPAIR * KT)
    )
    # vv[p, t*512 + b*64 + d] = V[b, t*128+p, d] (k-tile major)
    vv = np.ascontiguousarray(
        v16.reshape(B, NKT, KT, D).transpose(2, 1, 0, 3).reshape(128, NKT * B * D)
    )

    qts = []
    for c in range(NCORES):
        qc = q16[:, c * QBLK : (c + 1) * QBLK, :]  # [8, 512, 64]
        qt = np.ascontiguousarray(
            qc.transpose(0, 2, 1).reshape(NPAIR, 128, QBLK).transpose(1, 0, 2).reshape(128, NPAIR * QBLK)
        )
        qts.append(qt)
    return qts, kt, vv


def kernel(queries, keys, values):
    global LAST_RESULT
    from concourse.bass_utils import run_bass_kernel_spmd

    queries = np.asarray(queries, dtype=np.float32)
    keys = np.asarray(keys, dtype=np.float32)
    values = np.asarray(values, dtype=np.float32)

    nc = _get_nc()
    qts, kt, vv = _host_prep(queries, keys, values)
    in_maps = [{"qt": qts[c], "kt": kt, "vv": vv} for c in range(NCORES)]

    res = run_bass_kernel_spmd(
        nc,
        in_maps,
        list(range(NCORES)),
        trace=TRACE,
        **TRACE_KWARGS,
    )
    LAST_RESULT = res

    out = np.empty((B, S, D), dtype=np.float32)
    for c in range(NCORES):
        o = res.results[c]["out"]  # [4, 128, 512] = [j, (b%2)*64+d, q]
        out[:, c * QBLK : (c + 1) * QBLK, :] = (
            o.reshape(B, D, QBLK).transpose(0, 2, 1)
        )
    return out
